# revision 29
# baseline (speedup 1.0000x reference)
"""Trainium2 Bass kernel for grouped-query causal self-attention.

Problem shapes (hardcoded): x [8,1024,1024] f32, W_attn [6144,1024] f32,
W_proj [1024,4096] f32. 16 heads, head_dim 64, 4 query sets sharing one K/V.

Sharding: data parallel over batch — one batch element per NeuronCore (8 cores).
No collectives needed.

Per-core algorithm (everything "transposed" = [feature, token] layout so no
on-device transposes are needed; x is pre-transposed on the host):
  1. qkvT tiles = W_attn @ x^T   (stationary = W_attn^T tile, moving = x^T)
     -> K^T [1024f, 1024t], Q_g^T per set, and V in normal [t, f] layout.
  2. Attention per (set g, head PAIR j=(2j,2j+1)), computed transposed:
     the two heads of a pair live in partitions 0-63 / 64-127 of the same
     kt/qt tile, so their 64-contraction QK^T matmuls run CONCURRENTLY on
     the PE via row tiling (tile_position (0,0) and (64,0)) — 2x QK rate.
        S^T[k, q] = K_tile^T-stationary @ Q^T-moving   (contraction = 64)
        P^T = exp(S^T * scale)   one ACT op per k-tile covers both heads
        causal: zero the 128x128 diagonal triangle of P^T post-exp with
        gpsimd affine_select (idle engine; no PE mask matmuls)
        y_aug^T[d, q] = V_aug-stationary @ P^T-moving  (V augmented with a
        ones column -> row 64 of y_aug^T = softmax denominator for free)
        normalize: denominator row spread to [64,8] lanes via SBUF->SBUF DMA,
        reciprocal_approx_fast, DRAM hop for partition-broadcast, multiply.
  3. out = combined @ W_proj^T accumulated over sets in bf16 (osb), moving =
     W_proj^T streamed from DRAM in batched [128,8,512] loads.
dtypes: bf16 operands for matmuls (fp32 PSUM accumulate), fp32 softmax
denominator path, bf16 output accumulation (host converts to f32).
"""

import math

import ml_dtypes
import numpy as np

import concourse.bacc as bacc
import concourse.bass as bass
import concourse.mybir as mybir
import concourse.tile as tile
from concourse.bass_utils import run_bass_kernel_spmd

BF16 = ml_dtypes.bfloat16

B, T, C = 8, 1024, 1024
NH, HD, NQS = 16, 64, 4
SCALE = 1.0 / math.sqrt(HD)
NT = T // 128  # token tiles
NCH = C // 128  # channel tiles
KOFF = NQS * C  # 4096: K rows in W_attn
VOFF = (NQS + 1) * C  # 5120: V rows in W_attn

_CACHE = {}
LAST = {}  # exec_time_ns etc for test harness


def _build():
    f32 = mybir.dt.float32
    bf16 = mybir.dt.bfloat16
    EXP = mybir.ActivationFunctionType.Exp

    nc = bacc.Bacc()
    xT = nc.declare_dram_parameter("xT", [C, T], bf16, isOutput=False)
    waT = nc.declare_dram_parameter("waT", [C, 6 * C], bf16, isOutput=False)
    wpT = nc.declare_dram_parameter("wpT", [NQS * C, C], bf16, isOutput=False)
    vonesD = nc.declare_dram_parameter("vones", [128, NH, 1], bf16, isOutput=False)
    identD = nc.declare_dram_parameter("ident", [128, 128], bf16, isOutput=False)
    cmaskD = nc.declare_dram_parameter("cmaskT", [128, 128], bf16, isOutput=False)
    out = nc.declare_dram_parameter("out", [T, C], bf16, isOutput=True)
    # DRAM bounce rows for the partition-broadcast of softmax reciprocals.
    rscr = nc.dram_tensor("rscr", [128, 512], bf16)

    with tile.TileContext(nc) as tc:
        with (
            tc.tile_pool(name="res", bufs=1) as res,
            tc.tile_pool(name="wa", bufs=3) as wa_pool,
            tc.tile_pool(name="wp", bufs=2) as wp_pool,
            tc.tile_pool(name="pt", bufs=4) as pt_pool,
            tc.tile_pool(name="yab", bufs=4) as yab_pool,
            tc.tile_pool(name="small", bufs=4) as small_pool,
            tc.tile_pool(name="qp", bufs=1, space="PSUM") as qp_pool,
            tc.tile_pool(name="op", bufs=1, space="PSUM") as op_pool,
            tc.tile_pool(name="ypl", bufs=2, space="PSUM") as ypl,
            tc.tile_pool(name="ptmp", bufs=2, space="PSUM") as ptmp,
        ):
            xt = [res.tile([128, T], bf16, tag=f"xt{i}", name=f"xt{i}") for i in range(NCH)]
            kt = [res.tile([128, T], bf16, tag=f"kt{i}", name=f"kt{i}") for i in range(NCH)]
            # [NH, 65] V_aug per head + 64 pad cols so every head h has a full
            # 128-wide stationary slice at offset h*65 (FWL-eligible LDW)
            vt = [res.tile([128, NH * (HD + 1) + 64], bf16, tag=f"vt{i}", name=f"vt{i}") for i in range(NT)]
            # double-buffered across query-set parity so q-proj(g+1) overlaps
            # attention(g), and out-proj(g) overlaps attention(g+1)
            qts = [
                [res.tile([128, T], bf16, tag=f"qt{p}_{i}", name=f"qt{p}_{i}") for i in range(NCH)]
                for p in range(2)
            ]
            yts = [
                [res.tile([128, T], bf16, tag=f"yt{p}_{i}", name=f"yt{p}_{i}") for i in range(NCH)]
                for p in range(2)
            ]
            osb = [res.tile([128, C], bf16, tag=f"osb{i}", name=f"osb{i}") for i in range(NT)]

            ident = res.tile([128, 128], bf16, tag="ident", name="ident")
            cmask = res.tile([128, 128], bf16, tag="cmask", name="cmask")
            nc.sync.dma_start(out=ident, in_=identD[:, :])
            nc.sync.dma_start(out=cmask, in_=cmaskD[:, :])
            vtv = [
                vt[tt][:, 0 : NH * (HD + 1)].rearrange("p (a b) -> p a b", b=HD + 1)
                for tt in range(NT)
            ]
            for tt in range(NT):
                nc.vector.memset(vt[tt][:, NH * (HD + 1) :], 0.0)
                nc.sync.dma_start(out=vtv[tt][:, :, HD : HD + 1], in_=vonesD[:, :, :])
            for i in range(NCH):
                nc.sync.dma_start(out=xt[i], in_=xT[i * 128 : (i + 1) * 128, :])

            def project_T(dst, fbase, tag, pool):
                """dst[i][f_local, t] = (x @ W_attn.T).T rows fbase..fbase+1024."""
                for fg in range(2):  # 512-wide feature groups
                    f0 = fbase + fg * 512
                    w = wa_pool.tile([128, NCH, 512], bf16, tag="wa", name=f"wa_{tag}_{fg}")
                    nc.sync.dma_start(
                        out=w,
                        in_=waT[:, f0 : f0 + 512].rearrange("(a p) c -> p a c", p=128),
                    )
                    for tc2 in range(2):
                        for ftl in range(4):
                            ps = pool.tile(
                                [128, 512], f32, tag="psproj",
                                name=f"ps_{tag}_{fg}_{tc2}_{ftl}",
                            )
                            for ct in range(NCH):
                                nc.tensor.matmul(
                                    ps,
                                    w[:, ct, ftl * 128 : (ftl + 1) * 128],
                                    xt[ct][:, tc2 * 512 : (tc2 + 1) * 512],
                                    start=(ct == 0),
                                    stop=(ct == NCH - 1),
                                )
                            fti = fg * 4 + ftl
                            nc.vector.tensor_copy(
                                dst[fti][:, tc2 * 512 : (tc2 + 1) * 512], ps
                            )

            project_T(kt, KOFF, "k", qp_pool)

            # V in [token, feature] layout, features interleaved with a ones
            # column every 64 (each head's stationary V_aug slice is [128, 65]).
            for fg in range(2):
                f0 = VOFF + fg * 512
                w = wa_pool.tile([128, NCH, 512], bf16, tag="wa", name=f"wav_{fg}")
                nc.sync.dma_start(
                    out=w,
                    in_=waT[:, f0 : f0 + 512].rearrange("(a p) c -> p a c", p=128),
                )
                for tt in range(NT):
                    ps = op_pool.tile([128, 512], f32, tag="psop", name=f"psv_{fg}_{tt}")
                    for ct in range(NCH):
                        nc.tensor.matmul(
                            ps,
                            xt[ct][:, tt * 128 : (tt + 1) * 128],
                            w[:, ct, :],
                            start=(ct == 0),
                            stop=(ct == NCH - 1),
                        )
                    nc.vector.tensor_copy(
                        vtv[tt][:, fg * 8 : (fg + 1) * 8, 0:HD],
                        ps.rearrange("p (a b) -> p a b", b=HD),
                    )

            for g in range(NQS):
                qt = qts[g % 2]
                yt = yts[g % 2]
                project_T(qt, g * C, f"q{g}", qp_pool)

                for j in range(NH // 2):  # head pairs (2j, 2j+1)
                    for qc in range(2):  # 512-wide query chunks
                        nkt = 4 * qc + 4
                        yps = [
                            ypl.tile([128, 512], f32, tag="ypl", name=f"yp{g}_{j}_{qc}_{hh}")
                            for hh in range(2)
                        ]
                        # software-pipelined: QK(k2)+exp(k2) emitted one step
                        # ahead of PV(k2-1) so the PE never waits on ACT
                        pts = [None] * nkt
                        geom = []
                        for k2 in range(nkt):
                            qlo = max(qc * 512, k2 * 128)
                            wdt = qc * 512 + 512 - qlo
                            geom.append((qlo, wdt))
                        for k2 in range(nkt + 1):
                            if k2 < nkt:
                                qlo, wdt = geom[k2]
                                diag = k2 * 128 >= qc * 512
                                sp = ptmp.tile(
                                    [128, 1024], f32, tag="ptmp",
                                    name=f"sp{g}_{j}_{qc}_{k2}",
                                )
                                for hh in range(2):
                                    nc.tensor.matmul(
                                        sp[:, hh * 512 : hh * 512 + wdt],
                                        kt[j][hh * 64 : hh * 64 + 64, k2 * 128 : (k2 + 1) * 128],
                                        qt[j][hh * 64 : hh * 64 + 64, qlo : qlo + wdt],
                                        start=True,
                                        stop=not diag,
                                    )
                                if diag:
                                    # additive causal mask (0 / -1e30) on the
                                    # 128x128 diagonal blocks, applied on the
                                    # PE so the exp/PV deps stay single-engine
                                    for hh in range(2):
                                        nc.tensor.matmul(
                                            sp[:, hh * 512 : hh * 512 + 128],
                                            cmask,
                                            ident,
                                            start=False,
                                            stop=True,
                                            skip_group_check=True,
                                        )
                                pt = pt_pool.tile(
                                    [128, 2, 512], bf16, tag="pt",
                                    name=f"pt{g}_{j}_{qc}_{k2}",
                                )
                                nc.scalar.activation(
                                    pt[:, :, 0:wdt],
                                    sp.rearrange("p (a c) -> p a c", a=2)[:, :, 0:wdt],
                                    EXP,
                                    bias=0.0,
                                    scale=SCALE,
                                )
                                pts[k2] = pt
                            if k2 > 0:
                                qlo, wdt = geom[k2 - 1]
                                off = qlo - qc * 512
                                for hh in range(2):
                                    h = 2 * j + hh
                                    # 128-wide stationary slice (V_aug of head
                                    # h + spillover) -> FWL background load;
                                    # out rows 65-127 are unused garbage
                                    nc.tensor.matmul(
                                        yps[hh][:, off : off + wdt],
                                        vt[k2 - 1][:, h * (HD + 1) : h * (HD + 1) + 128],
                                        pts[k2 - 1][:, hh, 0:wdt],
                                        start=(k2 - 1 == 0),
                                        stop=(k2 - 1 == nkt - 1),
                                    )
                        for hh in range(2):
                            yab = yab_pool.tile(
                                [65, 512], bf16, tag="yab", name=f"yab{g}_{j}_{qc}_{hh}"
                            )
                            nc.vector.tensor_copy(yab, yps[hh][0:65, :])
                            # spread the single-partition denominator row to
                            # [64, 8] lanes (SBUF->SBUF DMA), reciprocal,
                            # then DRAM hop to partition-broadcast it back
                            den64 = small_pool.tile(
                                [64, 8], bf16, tag="den64", name=f"den{g}_{j}_{qc}_{hh}"
                            )
                            nc.sync.dma_start(out=den64, in_=yab[64:65, :])
                            rec64 = small_pool.tile(
                                [64, 8], bf16, tag="rec64", name=f"rec{g}_{j}_{qc}_{hh}"
                            )
                            with nc.allow_low_precision("bf16 softmax recip ok at 2e-2 tol"):
                                nc.vector.reciprocal(out=rec64, in_=den64)
                            ci = ((g * 8 + j) * 2 + qc) * 2 + hh
                            drow = rscr[ci : ci + 1, :]
                            nc.sync.dma_start(
                                out=drow.rearrange("a (b c) -> (a b) c", b=64), in_=rec64
                            )
                            bcst = small_pool.tile(
                                [64, 512], bf16, tag="bcst", name=f"bcst{g}_{j}_{qc}_{hh}"
                            )
                            nc.sync.dma_start(
                                out=bcst,
                                in_=bass.AP(
                                    tensor=drow.tensor,
                                    offset=drow.offset,
                                    ap=[[0, 64]] + drow.ap[1:],
                                ),
                            )
                            nc.vector.tensor_mul(
                                yt[j][hh * 64 : hh * 64 + 64, qc * 512 : qc * 512 + 512],
                                yab[0:64, :],
                                bcst,
                            )

                # projection for this set, accumulated into osb (bf16)
                for cc in range(2):
                    wp = wp_pool.tile([128, NCH, 512], bf16, tag="wp", name=f"wp{g}_{cc}")
                    nc.sync.dma_start(
                        out=wp,
                        in_=wpT[
                            g * C : (g + 1) * C, cc * 512 : (cc + 1) * 512
                        ].rearrange("(a p) c -> p a c", p=128),
                    )
                    for tt in range(NT):
                        # final set: attention is over, so ypl's banks are idle
                        # -> alternate pools to unserialize the last projection
                        pool2 = ypl if (g == NQS - 1 and tt % 2 == 1) else op_pool
                        tg = "ypl" if pool2 is ypl else "psop"
                        ps = pool2.tile(
                            [128, 512], f32, tag=tg, name=f"psp{g}_{cc}_{tt}"
                        )
                        for ftl in range(NCH):
                            nc.tensor.matmul(
                                ps,
                                yt[ftl][:, tt * 128 : (tt + 1) * 128],
                                wp[:, ftl, :],
                                start=(ftl == 0),
                                stop=(ftl == NCH - 1),
                            )
                        dst = osb[tt][:, cc * 512 : (cc + 1) * 512]
                        if g == 0:
                            nc.vector.tensor_copy(dst, ps)
                        else:
                            nc.vector.tensor_add(dst, dst, ps)

            for tt in range(NT):
                nc.sync.dma_start(out=out[tt * 128 : (tt + 1) * 128, :], in_=osb[tt])

    nc.compile()
    return nc


def kernel(x, W_attn, W_proj, _trace=False):
    if "nc" not in _CACHE:
        _CACHE["nc"] = _build()
    nc = _CACHE["nc"]

    xT = np.ascontiguousarray(np.transpose(np.asarray(x, np.float32), (0, 2, 1))).astype(BF16)
    waT = np.ascontiguousarray(np.asarray(W_attn, np.float32).T).astype(BF16)
    wpT = np.ascontiguousarray(np.asarray(W_proj, np.float32).T).astype(BF16)
    vones = np.ones((128, NH, 1), np.float32).astype(BF16)
    ii = np.arange(128)
    ident = np.eye(128, dtype=np.float32).astype(BF16)
    # lhsT for the mask matmul: out[k,q] = cmaskT[q,k] = 0 if q>=k else -1e30
    cmaskT = (
        np.where(ii[:, None] >= ii[None, :], 0.0, -1e30)
        .astype(np.float32)
        .astype(BF16)
    )

    in_maps = [
        {"xT": xT[b], "waT": waT, "wpT": wpT, "vones": vones, "ident": ident,
         "cmaskT": cmaskT}
        for b in range(B)
    ]
    res = run_bass_kernel_spmd(nc, in_maps, core_ids=list(range(B)), trace=_trace)
    LAST["exec_time_ns"] = res.exec_time_ns
    LAST["mean_exec_time_ns"] = res.mean_exec_time_ns
    LAST["results"] = res
    return np.stack([res.results[b]["out"] for b in range(B)]).astype(np.float32)
